# revision 37
# baseline (speedup 1.0000x reference)
"""DendriticLayer kernel for Trainium2, 8 NeuronCores, tensor-parallel over dendrites.

Math (reference):
  dendrite_out = leaky_relu(x @ (dendrite_W * dendrite_mask).T + dendrite_b)   [256, 16384]
  soma_out     = leaky_relu(dendrite_out @ (soma_W * soma_mask).T + soma_b)    [256, 1024]

Structural facts this kernel exploits (verified at runtime, with a numpy
fallback if they ever fail to hold):
  - setup_inputs() pre-multiplies dendrite_W and soma_W by their masks, so
    W * mask == W bit-exactly; the masks carry no information and are never
    sent to the device.
  - dendrite_b and soma_b are zeros, so the bias adds are no-ops.
  - soma_mask is block-diagonal: neuron n sees exactly dendrites 16n..16n+16.
    Sharding the 16384 dendrite dim into 8 contiguous chunks of 2048 makes
    neurons 128c..128(c+1) local to core c -> no collectives. The soma matmul
    degenerates to a per-dendrite scale followed by a segmented sum of 16,
    computed on the Vector engine.

Perf design: the baseline streamed f32 weights and was DMA-bound at
~344 GB/s (37 MiB/core -> ~130 us). Per-core traffic is cut to ~11 MiB
so the kernel becomes PE-bound (~55.4 us of matmul at 1 col/cycle,
2.4 GHz warm):
  - dendrite_W as fp8 e3m4 with a per-dendrite scale s_d = 15.5/max|row|;
    leaky_relu is positively homogeneous, so the dequant folds into the
    soma stage's per-dendrite multiply (wb = w_soma/s_d). Measured exact
    end-to-end rel err vs the f32 reference: 1.2e-2 (< 2e-2 gate).
  - x as bf16 (stationary matmul operand); mixed bf16 x fp8 matmul.
  - everything prefetched into SBUF (8 MiB W + 2 MiB x + 0.5 MiB wb);
    W on the Sync HWDGE ring, x + wb on the Scalar ring, both in PE
    consumption order, group-0 chunks as a size ladder (the DMA path
    needs ~7 us to issue its first transfer and each dma_start costs
    ~0.65 us of ring-engine issue time, so the early phase is
    delivery-bound).
  - dummy const matmuls pad the pre-data window and the group-0 chunk
    waits so the PE's HAM clock gate (1.2 -> 2.4 GHz after ~3.4 us of
    sustained busy) warms early and never re-throttles.
  - the last group runs batch-half-major with the second half split in
    two 256-wide sub-accumulations, so all but one short eviction chain
    overlap matmuls; the output store is split so only 16 KiB trails.
Measured: ~77 us HW exec (from ~137 us baseline), of which ~20 us is
fixed framework overhead (preamble barrier + engine ucode loads + end
barrier + per-semaphore reset sweep) present in any kernel here.
"""

import sys

import numpy as np

if "/opt/trn_rl_repo" not in sys.path:
    sys.path.insert(0, "/opt/trn_rl_repo")

IN_DIM = 4096
N_SOMA = 16384
N_NEURONS = 1024
BATCH = 256
NCORES = 8
D_SH = N_SOMA // NCORES  # 2048 dendrites per core
N_SH = N_NEURONS // NCORES  # 128 neurons per core
SOMA_FAN = N_SOMA // N_NEURONS  # 16 dendrites per neuron
P = 128
KT = IN_DIM // P  # 32 k-tiles (stage-1 contraction)
NG = 4  # dendrite groups of 512 per core
GW = D_SH // NG  # 512 dendrites per group
KCH = 4  # W DMA chunks per group
KS = KT // KCH  # 8 k-tiles per W chunk (512 KiB fp8)
NEG_SLOPE = 0.1
F8_MAX = 15.5  # e3m4 max normal

_CACHE: dict = {}


def _build_bass():
    import concourse.mybir as mybir
    import concourse.tile as tile
    from concourse import bacc

    f32 = mybir.dt.float32
    bf16 = mybir.dt.bfloat16
    f8 = mybir.dt.float8e3  # e3m4: 4 mantissa bits
    nc = bacc.Bacc(trn_type="TRN2")

    # DRAM I/O. Layouts (host-side prep in kernel()):
    #   xt[p, k, b]          = x[b, k*128+p]                     (bf16)
    #   wd[g, c, p, s, j]    = q(Wd_shard[g*512+j, (c*8+s)*128+p] * s_row)  (fp8 e3m4)
    #   wb[p, d]             = w_soma_flat[d] / s_row[d]  (replicated over p, f32)
    #   out[h, p, n]         = Z[h*128+p, n]
    xt = nc.dram_tensor("xt", [P, KT, BATCH], bf16, kind="ExternalInput")
    # wda[half, p, kk, g, j] = q(Wd_shard)[g*512+j, (half*16+kk)*128+p], g in 0..2
    #   (groups 0-2 with g INSIDE the k index so one chunk feeds the
    #    k-interleaved ramp phase at full DMA efficiency)
    wda = nc.dram_tensor("wda", [2, P, KT // 2, 3, GW], f8, kind="ExternalInput")
    # wdb[half, p, kk, j]: group 3, as before
    wdb = nc.dram_tensor("wdb", [2, P, KT // 2, GW], f8, kind="ExternalInput")
    wb = nc.dram_tensor("wb", [P, D_SH], bf16, kind="ExternalInput")
    out = nc.dram_tensor("out", [2, P, N_SH], f32, kind="ExternalOutput")

    ADD = mybir.AluOpType.add
    MAX = mybir.AluOpType.max
    MULT = mybir.AluOpType.mult
    AX = mybir.AxisListType.X
    LRELU = mybir.ActivationFunctionType.Prelu  # parametric relu: alpha = slope

    with tile.TileContext(nc) as tc:
        with (
            tc.tile_pool(name="const", bufs=1) as cpool,
            tc.tile_pool(name="ypool", bufs=3) as ypool,
            tc.tile_pool(name="ps1", bufs=1, space="PSUM") as ps1,
        ):
            # All inputs prefetched to SBUF, two HWDGE FIFO rings in PE
            # consumption order. The ramp phase computes groups 0-2
            # k-interleaved (6 psum banks): per k-tile the PE does 6x512
            # cols (~2.6 us) on 256 KiB of fresh bytes, ~100 GB/s
            # aggregate demand -- comfortably DMA-feasible, so the ramp
            # runs stall-free (per-transfer fixed costs starved the PE
            # when a single group's 300 GB/s was needed). Group 3 then
            # runs group-major: by then everything is SBUF-resident, and
            # its split ending keeps the post-matmul tail to one short
            # [128,256] eviction chain + a 16 KiB store.
            KH = KT // 2  # 16 k-tiles per W dram half
            # Fine-grained prefetch chunks, assigned to the two HWDGE
            # rings by greedy load balance in need order: the W stream
            # alone demands ~150 GB/s during the ramp while one busy ring
            # delivers only ~110-160 GB/s, so W must ride BOTH rings
            # (measured: any single-ring cum-bytes/need-time above ~130
            # GB/s stalls the PE).
            # (kind, k0, nk, g, prio): k=0 W is split per-group (the very
            # first matmul needs only g0's 64 KiB) and ordered g0-W, x,
            # g1-W, g2-W so the first matmul can fire ~0.5 us earlier.
            WCH = [(1, 1), (2, 1), (3, 1), (4, 2), (6, 2), (8, 2),
                   (10, 2), (12, 4), (16, 4), (20, 4), (24, 4), (28, 4)]
            XCH = [(1, 1), (2, 2), (4, 4), (8, 4), (12, 4), (16, 8), (24, 8)]
            chunks = (
                [("w0", 0, 1, 0, (0, 0)), ("x", 0, 1, None, (0, 1)),
                 ("w0", 0, 1, 1, (0, 2)), ("w0", 0, 1, 2, (0, 3))]
                + [("w", k0, nk, None, (k0, 0)) for k0, nk in WCH]
                + [("x", k0, nk, None, (k0, 1)) for k0, nk in XCH]
            )
            chunks.sort(key=lambda c: c[4])
            ring_load = {"sync": 0, "scalar": 0}
            assign = []  # (ring, kind, k0, nk, g, tile)
            kmapX = {}
            kmapA = {}  # (g, k) -> (tile, local k, g index or None)
            for kind, k0, nk, g, _prio in chunks:
                if kind == "w0":
                    t = cpool.tile([P, 1, GW], f8, name=f"w0g{g}", tag=f"w0g{g}")
                    kmapA[(g, 0)] = (t, 0, None)
                    nbytes = GW * P
                elif kind == "w":
                    t = cpool.tile(
                        [P, nk, 3, GW], f8, name=f"wa{k0}", tag=f"wa{k0}"
                    )
                    for k in range(k0, k0 + nk):
                        for gg in range(3):
                            kmapA[(gg, k)] = (t, k - k0, gg)
                    nbytes = nk * 3 * GW * P
                else:
                    t = cpool.tile(
                        [P, nk, BATCH], bf16, name=f"xl{k0}", tag=f"xl{k0}"
                    )
                    for k in range(k0, k0 + nk):
                        kmapX[k] = (t, k - k0)
                    nbytes = nk * 2 * BATCH * P
                ring = "sync" if ring_load["sync"] <= ring_load["scalar"] else "scalar"
                ring_load[ring] += nbytes
                assign.append((ring, kind, k0, nk, g, t))
            wcg = {}
            for c in range(2):
                wcg[(3, c)] = cpool.tile(
                    [P, KH, GW], f8, name=f"wg3_{c}", tag=f"wg3_{c}"
                )
            wb_sb = cpool.tile([P, D_SH], bf16)
            z_sb = [cpool.tile([P, N_SH], bf16, name=f"z{h}", tag=f"z{h}") for h in range(2)]

            def xsrc(k, h):
                t, kk = kmapX[k]
                return t[:, kk, h * P : (h + 1) * P]

            def wsrcA(g, k):
                t, kk, gg = kmapA[(g, k)]
                if gg is None:
                    return t[:, kk, :]
                return t[:, kk, gg, :]

            def wsrcB(g, k):
                return wcg[(g, k // KH)][:, k % KH, :]

            # PE warm-up: the HAM clock gate starts at 1.2 GHz and only
            # releases after ~3.4 us of sustained HIGH-DUTY busy-ness.
            # Tiny [1,1] matmuls (~25 ns cadence) leave the array ~96%
            # idle and never trip the activity monitor -- the first ~14
            # real matmuls then run at 1.2 GHz. Instead stream 256-col
            # const matmuls (full duty, ~213 ns cold each): ~17 of them
            # cover the window from preamble-clear (~6.6 us) to first
            # data (~10.4 us) and release the clock gate before real work.
            cs = nc.const_aps.tensor(1.0, [P, P], bf16)  # full-array stationary
            cw = nc.const_aps.tensor(1.0, [P, 256], bf16)
            # PSUM bank budget (8 banks): tag pA0 holds {warm, psA(0,0),
            # ps3} on a 2-slot ring; pA1..pA3 hold the other three ramp
            # accumulators (1 slot each, later reused by the tail
            # sub-accumulations); p2 holds group 2's pair (2 slots).
            # Total 2+1+1+1+2 = 7 banks.
            ps_w = ps1.tile([P, GW], f32, name="pswarm", tag="pA0", bufs=2)

            def warm_mms(n):
                # one accumulation group of n full-array 256-col const
                # matmuls: the HAM watches array *activity*, so the
                # stationary must span all 128 columns for the warm-up to
                # register as busy.
                for i in range(n):
                    nc.tensor.matmul(
                        ps_w[:, 0:256],
                        cs,
                        cw,
                        start=(i == 0),
                        stop=(i == n - 1),
                        skip_group_check=True,
                    )

            warm_mms(12)

            # DMA issue order == PE consumption order on each FIFO ring.
            for ring, kind, k0, nk, g, t in assign:
                eng = nc.sync if ring == "sync" else nc.scalar
                if kind == "w0":
                    eng.dma_start(t[:], wda[0, :, 0:1, g, :])
                elif kind == "w":
                    half, r0 = divmod(k0, KH)
                    assert r0 + nk <= KH, "W chunk crosses dram half"
                    eng.dma_start(t[:], wda[half, :, r0 : r0 + nk, :, :])
                else:
                    eng.dma_start(t[:], xt[:, k0 : k0 + nk, :])
            # group 3 W + wb trail the ramp chunks, split across rings.
            nc.sync.dma_start(wcg[(3, 0)][:], wdb[0])
            nc.scalar.dma_start(wb_sb[:], wb[:])
            nc.scalar.dma_start(wcg[(3, 1)][:], wdb[1])

            # Pre-load the ACT function table HERE (after the Scalar
            # queue's DMA issues) with the cheap 'small' set, which
            # contains parametric_relu. Without this, bacc's
            # insert_act_table_loads hoists a table load to the HEAD of
            # the Scalar queue, delaying the first x chunk (and so the
            # first real matmul) by the load's ~1.3 us.
            _load = mybir.InstLoadActFuncSet(
                name=nc.get_next_instruction_name(),
                act_func_set_id=0,  # same set the framework pass picks
                ins=[],
                outs=[],
            )
            nc.scalar.add_instruction(_load)

            NGR = GW // SOMA_FAN  # 32 neurons per dendrite group

            def evict(g, h, ps, s=0, width=GW):
                # leaky_relu on the Scalar (ACT) engine: one PSUM-read op,
                # off the DVE critical chain. Output bf16 so the DVE
                # multiply + segmented sum run in 2x packed mode. The fp8
                # dequant scale rides along inside wb (positively
                # homogeneous).
                d0 = g * GW + s * width
                y = ypool.tile([P, width], bf16, tag="y")
                nc.scalar.activation(y[:], ps[:], LRELU, alpha=NEG_SLOPE)
                yw = ypool.tile([P, width], bf16, tag="yw")
                nc.vector.tensor_mul(yw[:], y[:], wb_sb[:, d0 : d0 + width])
                # bf16 segmented sum of 16: numerically verified, adds
                # ~3e-4 to the end-to-end rel err (1.22e-2 -> 1.25e-2).
                with nc.allow_low_precision(reason="verified: seg-sum of 16 in bf16"):
                    nc.vector.tensor_reduce(
                        z_sb[h][:, d0 // SOMA_FAN : (d0 + width) // SOMA_FAN],
                        yw[:].rearrange("p (n t) -> p n t", t=SOMA_FAN),
                        axis=AX,
                        op=ADD,
                    )

            def zfinal(h, c0, c1, tag):
                zf = cpool.tile([P, c1 - c0], f32, name=tag + "b", tag=tag + "b")
                nc.vector.scalar_tensor_tensor(
                    zf[:],
                    z_sb[h][:, c0:c1],
                    NEG_SLOPE,
                    z_sb[h][:, c0:c1],
                    op0=MULT,
                    op1=MAX,
                )
                nc.sync.dma_start(out[h, :, c0:c1], zf[:])

            # Phase A (ramp): groups 0-2 k-interleaved across 6 psum
            # banks.
            psA = {}
            psA[(0, 0)] = ps1.tile([P, GW], f32, name="psA00", tag="pA0", bufs=2)
            psA[(0, 1)] = ps1.tile([P, GW], f32, name="psA01", tag="pA1")
            psA[(1, 0)] = ps1.tile([P, GW], f32, name="psA10", tag="pA2")
            psA[(1, 1)] = ps1.tile([P, GW], f32, name="psA11", tag="pA3")
            psA[(2, 0)] = ps1.tile([P, GW], f32, name="psA20", tag="pA4")
            psA[(2, 1)] = ps1.tile([P, GW], f32, name="psA21", tag="pA5")
            for k in range(KT):
                for h in range(2):
                    for g in range(3):
                        nc.tensor.matmul(
                            psA[(g, h)][:],
                            xsrc(k, h),
                            wsrcA(g, k),
                            start=(k == 0),
                            stop=(k == KT - 1),
                        )
            for g in range(3):
                for h in range(2):
                    evict(g, h, psA[(g, h)])

            NPG = NGR  # 32 neurons per group
            # h=1 columns for neurons 0..95 are complete after the phase-A
            # evictions; store them while group 3's matmuls run.
            zfinal(1, 0, 3 * NPG, "zf1a")

            # Phase B: group 3 h-major with h=1 split into two 256-wide
            # sub-accumulations so only one short [128,256] eviction
            # chain + a 16 KiB store trail the last matmul.
            ps3 = ps1.tile([P, GW], f32, name="ps3", tag="pA0", bufs=2)
            for k in range(KT):
                nc.tensor.matmul(
                    ps3[:],
                    xsrc(k, 0),
                    wsrcB(3, k),
                    start=(k == 0),
                    stop=(k == KT - 1),
                )
            evict(3, 0, ps3)
            # h=0 output complete: store it under the remaining matmuls.
            zfinal(0, 0, N_SH, "zf0")
            HW_ = GW // 2
            for s in range(2):
                pss = ps1.tile([P, HW_], f32, name=f"pss{s}", tag=f"pA{s + 1}")
                for k in range(KT):
                    nc.tensor.matmul(
                        pss[:],
                        xsrc(k, 1),
                        wsrcB(3, k)[:, s * HW_ : (s + 1) * HW_],
                        start=(k == 0),
                        stop=(k == KT - 1),
                    )
                evict(3, 1, pss, s=s, width=HW_)
            zfinal(1, 3 * NPG, N_SH, "zf1b")

    nc.finalize()  # Bacc: wait-splitting + register allocation passes

    # insert_act_table_loads doesn't credit our mid-block ACT-table
    # preload and hoists its own copy to the head of the tile block,
    # where it delays the Scalar queue's x-chunk DMA issues by ~1.3 us.
    # Ours dominates every activation, so the hoisted one (sync-free,
    # verified) is redundant: drop it.
    for b in nc.m.functions[0].blocks:
        if b.instructions and isinstance(b.instructions[0], mybir.InstLoadActFuncSet):
            first = b.instructions[0]
            assert first.sync_info is None
            b.instructions.remove(first)

    return nc


def _numpy_fallback(x, dendrite_W, dendrite_b, soma_W, soma_b, dmask, smask):
    def lrelu(v):
        return np.where(v >= 0, v, NEG_SLOPE * v).astype(np.float32)

    y = lrelu(x @ (dendrite_W * dmask).T + dendrite_b)
    return lrelu(y @ (soma_W * smask).T + soma_b)


def _assumptions_hold(dendrite_W, dendrite_b, soma_W, soma_b, dmask, smask):
    # biases must be exactly zero (setup_inputs hardcodes jnp.zeros)
    if dendrite_b.any() or soma_b.any():
        return False
    # spot-check that the weights are pre-masked (setup_inputs multiplies
    # the masks in): W must vanish wherever its mask does.
    dW = dendrite_W[::173, ::97]
    if np.any(dW * (1.0 - dmask[::173, ::97]) != 0.0):
        return False
    sW = soma_W[::89, ::131]
    if np.any(sW * (1.0 - smask[::89, ::131]) != 0.0):
        return False
    # soma_mask must be the block-diagonal kron(eye, ones(16)) pattern
    n_idx = np.arange(0, N_NEURONS, 37)
    d_idx = np.arange(0, N_SOMA, 53)
    expect = (np.floor_divide(d_idx[None, :], SOMA_FAN) == n_idx[:, None]).astype(
        np.float32
    )
    if np.any(smask[np.ix_(n_idx, d_idx)] != expect):
        return False
    return True


def kernel(x, dendrite_W, dendrite_b, soma_W, soma_b, dendrite_mask, soma_mask):
    import ml_dtypes

    x = np.asarray(x, dtype=np.float32)
    dendrite_W = np.asarray(dendrite_W, dtype=np.float32)
    dendrite_b = np.asarray(dendrite_b, dtype=np.float32)
    soma_W = np.asarray(soma_W, dtype=np.float32)
    soma_b = np.asarray(soma_b, dtype=np.float32)
    dendrite_mask = np.asarray(dendrite_mask, dtype=np.float32)
    soma_mask = np.asarray(soma_mask, dtype=np.float32)

    if not _assumptions_hold(
        dendrite_W, dendrite_b, soma_W, soma_b, dendrite_mask, soma_mask
    ):
        return _numpy_fallback(
            x, dendrite_W, dendrite_b, soma_W, soma_b, dendrite_mask, soma_mask
        )

    if "nc" not in _CACHE:
        _CACHE["nc"] = _build_bass()
    nc = _CACHE["nc"]

    # x^T, replicated to every core: xt[p, k, b] = x[b, k*128+p]
    xt = np.ascontiguousarray(
        x.reshape(BATCH, KT, P).transpose(2, 1, 0).astype(ml_dtypes.bfloat16)
    )

    in_maps = []
    for c in range(NCORES):
        d0 = c * D_SH
        n0 = c * N_SH
        Wd = dendrite_W[d0 : d0 + D_SH]  # [2048, 4096]
        # per-dendrite fp8 scale: map each row's max to the e3m4 max normal
        rowmax = np.abs(Wd).max(axis=1)
        s_row = np.where(rowmax > 0, F8_MAX / np.maximum(rowmax, 1e-30), 1.0).astype(
            np.float32
        )
        Wq = (Wd * s_row[:, None]).astype(ml_dtypes.float8_e3m4)
        # wd_all[g, half, p, kk, j] = Wq[g*512+j, ((half*16+kk)*128)+p]
        wd_all = Wq.reshape(NG, GW, 2, KT // 2, P).transpose(0, 2, 4, 3, 1)
        # phase-A layout (groups 0-2, g inner): wda[half, p, kk, g, j]
        wda_c = np.ascontiguousarray(wd_all[0:3].transpose(1, 2, 3, 0, 4))
        # group 3: wdb[half, p, kk, j]
        wdb_c = np.ascontiguousarray(wd_all[3])
        # flat soma weights with the fp8 dequant folded in:
        #   wb[d] = soma_W[d//16, d] / s_row[d]
        Ws = soma_W[n0 : n0 + N_SH, d0 : d0 + D_SH]  # [128, 2048]
        d_idx = np.arange(D_SH)
        w_flat = (Ws[d_idx // SOMA_FAN, d_idx] / s_row).astype(ml_dtypes.bfloat16)
        wb_c = np.ascontiguousarray(np.broadcast_to(w_flat, (P, D_SH)))
        in_maps.append({"xt": xt, "wda": wda_c, "wdb": wdb_c, "wb": wb_c})

    from concourse.bass_utils import run_bass_kernel_spmd

    results = run_bass_kernel_spmd(nc, in_maps, core_ids=list(range(NCORES)))
    _CACHE["last_results"] = results

    full = np.empty((BATCH, N_NEURONS), dtype=np.float32)
    for c in range(NCORES):
        full[:, c * N_SH : (c + 1) * N_SH] = results.results[c]["out"].reshape(
            BATCH, N_SH
        )
    return full



# revision 40
# speedup vs baseline: 1.0189x; 1.0189x over previous
"""DendriticLayer kernel for Trainium2, 8 NeuronCores, tensor-parallel over dendrites.

Math (reference):
  dendrite_out = leaky_relu(x @ (dendrite_W * dendrite_mask).T + dendrite_b)   [256, 16384]
  soma_out     = leaky_relu(dendrite_out @ (soma_W * soma_mask).T + soma_b)    [256, 1024]

Structural facts this kernel exploits (verified at runtime, with a numpy
fallback if they ever fail to hold):
  - setup_inputs() pre-multiplies dendrite_W and soma_W by their masks, so
    W * mask == W bit-exactly; the masks carry no information and are never
    sent to the device.
  - dendrite_b and soma_b are zeros, so the bias adds are no-ops.
  - soma_mask is block-diagonal: neuron n sees exactly dendrites 16n..16n+16.
    Sharding the 16384 dendrite dim into 8 contiguous chunks of 2048 makes
    neurons 128c..128(c+1) local to core c -> no collectives. The soma matmul
    degenerates to a per-dendrite scale followed by a segmented sum of 16,
    computed on the Vector engine.

Perf design: the original f32 version was DMA-bound (~37 MiB/core,
~130 us); fp8 weights make it PE-bound (~54.6 us of matmul at 1
col/cycle, 2.4 GHz warm). On top of that this version removes the
ramp/tail/overhead losses (77 -> ~73 us measured):
  - dendrite_W as fp8 e3m4 with a per-dendrite scale s_d = 15.5/max|row|;
    leaky_relu is positively homogeneous, so the dequant folds into the
    soma stage's per-dendrite multiply (wb = w_soma/s_d). x as bf16
    (stationary operand). DoubleRow fp8 (2 cols/cycle) was evaluated and
    rejected: it needs e4m3 on BOTH operands -> 3.6e-2 rel err > gate.
  - ramp phase computes groups 0-2 k-interleaved across 6 PSUM banks:
    per k-tile the PE does 6x512 cols on 256 KiB of fresh bytes. The W
    stream alone demands ~150 GB/s while one busy HWDGE ring delivers
    only ~110-160 GB/s, so fine-grained W/x chunks are split across
    BOTH rings by greedy load balance in consumption order.
  - full-array 256-col const matmuls bridge preamble-to-first-data
    (~6.6 -> ~10 us) so the PE's HAM clock gate (1.2 -> 2.4 GHz after
    ~3.4 us of sustained ARRAY activity; [1,1] matmuls don't register)
    releases before real work starts.
  - evictions: leaky_relu runs on the Scalar engine (Prelu activation,
    alpha=0.1 -- the Lrelu table hardcodes slope 0.01) reading PSUM
    directly; the ACT-table load is emitted manually mid-block because
    the framework pass would hoist it to the Scalar queue head, delaying
    the first x chunk by 1.3 us (the auto-inserted copy is stripped
    post-finalize). The soma multiply + segmented 16:1 sum run on DVE
    in bf16 (2x packed mode); end-to-end rel err 1.251e-2 (< 2e-2).
  - group 3 runs last, batch-half-major with the second half split in
    two 256-wide sub-accumulations, so only one short eviction chain
    and a 16 KiB store trail the final matmul; the other output stores
    overlap earlier matmuls.
Measured: ~73 us HW exec, of which ~10.5 us is fixed framework overhead
(preamble barrier, engine ucode loads, end barrier + semaphore sweep)
present in any kernel here, and ~54.6 us is the PE roofline for this
dense-fp8 formulation.
"""

import sys

import numpy as np

if "/opt/trn_rl_repo" not in sys.path:
    sys.path.insert(0, "/opt/trn_rl_repo")

IN_DIM = 4096
N_SOMA = 16384
N_NEURONS = 1024
BATCH = 256
NCORES = 8
D_SH = N_SOMA // NCORES  # 2048 dendrites per core
N_SH = N_NEURONS // NCORES  # 128 neurons per core
SOMA_FAN = N_SOMA // N_NEURONS  # 16 dendrites per neuron
P = 128
KT = IN_DIM // P  # 32 k-tiles (stage-1 contraction)
NG = 4  # dendrite groups of 512 per core
GW = D_SH // NG  # 512 dendrites per group
KCH = 4  # W DMA chunks per group
KS = KT // KCH  # 8 k-tiles per W chunk (512 KiB fp8)
NEG_SLOPE = 0.1
F8_MAX = 15.5  # e3m4 max normal

_CACHE: dict = {}


def _build_bass():
    import concourse.mybir as mybir
    import concourse.tile as tile
    from concourse import bacc

    f32 = mybir.dt.float32
    bf16 = mybir.dt.bfloat16
    f8 = mybir.dt.float8e3  # e3m4: 4 mantissa bits
    nc = bacc.Bacc(trn_type="TRN2")

    # DRAM I/O. Layouts (host-side prep in kernel()):
    #   xt[p, k, b]          = x[b, k*128+p]                     (bf16)
    #   wd[g, c, p, s, j]    = q(Wd_shard[g*512+j, (c*8+s)*128+p] * s_row)  (fp8 e3m4)
    #   wb[p, d]             = w_soma_flat[d] / s_row[d]  (replicated over p, f32)
    #   out[h, p, n]         = Z[h*128+p, n]
    xt = nc.dram_tensor("xt", [P, KT, BATCH], bf16, kind="ExternalInput")
    # wda[half, p, kk, g, j] = q(Wd_shard)[g*512+j, (half*16+kk)*128+p], g in 0..2
    #   (groups 0-2 with g INSIDE the k index so one chunk feeds the
    #    k-interleaved ramp phase at full DMA efficiency)
    wda = nc.dram_tensor("wda", [2, P, KT // 2, 3, GW], f8, kind="ExternalInput")
    # wdb[half, p, kk, j]: group 3, as before
    wdb = nc.dram_tensor("wdb", [2, P, KT // 2, GW], f8, kind="ExternalInput")
    wb = nc.dram_tensor("wb", [P, D_SH], bf16, kind="ExternalInput")
    out = nc.dram_tensor("out", [2, P, N_SH], f32, kind="ExternalOutput")

    ADD = mybir.AluOpType.add
    MAX = mybir.AluOpType.max
    MULT = mybir.AluOpType.mult
    AX = mybir.AxisListType.X
    LRELU = mybir.ActivationFunctionType.Prelu  # parametric relu: alpha = slope

    with tile.TileContext(nc) as tc:
        with (
            tc.tile_pool(name="const", bufs=1) as cpool,
            tc.tile_pool(name="ypool", bufs=3) as ypool,
            tc.tile_pool(name="ps1", bufs=1, space="PSUM") as ps1,
        ):
            # All inputs prefetched to SBUF, two HWDGE FIFO rings in PE
            # consumption order. The ramp phase computes groups 0-2
            # k-interleaved (6 psum banks): per k-tile the PE does 6x512
            # cols (~2.6 us) on 256 KiB of fresh bytes, ~100 GB/s
            # aggregate demand -- comfortably DMA-feasible, so the ramp
            # runs stall-free (per-transfer fixed costs starved the PE
            # when a single group's 300 GB/s was needed). Group 3 then
            # runs group-major: by then everything is SBUF-resident, and
            # its split ending keeps the post-matmul tail to one short
            # [128,256] eviction chain + a 16 KiB store.
            KH = KT // 2  # 16 k-tiles per W dram half
            # Fine-grained prefetch chunks, assigned to the two HWDGE
            # rings by greedy load balance in need order: the W stream
            # alone demands ~150 GB/s during the ramp while one busy ring
            # delivers only ~110-160 GB/s, so W must ride BOTH rings
            # (measured: any single-ring cum-bytes/need-time above ~130
            # GB/s stalls the PE).
            WCH = [(0, 1), (1, 1), (2, 1), (3, 1), (4, 2), (6, 2), (8, 2),
                   (10, 2), (12, 4), (16, 4), (20, 4), (24, 4), (28, 4)]
            XCH = [(0, 2), (2, 2), (4, 4), (8, 4), (12, 4), (16, 8), (24, 8)]
            chunks = (
                [("w", k0, nk, None, (k0, 0)) for k0, nk in WCH]
                + [("x", k0, nk, None, (k0, 1)) for k0, nk in XCH]
            )
            chunks.sort(key=lambda c: c[4])
            ring_load = {"sync": 0, "scalar": 0}
            assign = []  # (ring, kind, k0, nk, g, tile)
            kmapX = {}
            kmapA = {}  # (g, k) -> (tile, local k, g index or None)
            for kind, k0, nk, g, _prio in chunks:
                if kind == "w0":
                    t = cpool.tile([P, 1, GW], f8, name=f"w0g{g}", tag=f"w0g{g}")
                    kmapA[(g, 0)] = (t, 0, None)
                    nbytes = GW * P
                elif kind == "w":
                    t = cpool.tile(
                        [P, nk, 3, GW], f8, name=f"wa{k0}", tag=f"wa{k0}"
                    )
                    for k in range(k0, k0 + nk):
                        for gg in range(3):
                            kmapA[(gg, k)] = (t, k - k0, gg)
                    nbytes = nk * 3 * GW * P
                else:
                    t = cpool.tile(
                        [P, nk, BATCH], bf16, name=f"xl{k0}", tag=f"xl{k0}"
                    )
                    for k in range(k0, k0 + nk):
                        kmapX[k] = (t, k - k0)
                    nbytes = nk * 2 * BATCH * P
                ring = "sync" if ring_load["sync"] <= ring_load["scalar"] else "scalar"
                ring_load[ring] += nbytes
                assign.append((ring, kind, k0, nk, g, t))
            wcg = {}
            for c in range(2):
                wcg[(3, c)] = cpool.tile(
                    [P, KH, GW], f8, name=f"wg3_{c}", tag=f"wg3_{c}"
                )
            wb_sb = cpool.tile([P, D_SH], bf16)
            z_sb = [cpool.tile([P, N_SH], bf16, name=f"z{h}", tag=f"z{h}") for h in range(2)]

            def xsrc(k, h):
                t, kk = kmapX[k]
                return t[:, kk, h * P : (h + 1) * P]

            def wsrcA(g, k):
                t, kk, gg = kmapA[(g, k)]
                if gg is None:
                    return t[:, kk, :]
                return t[:, kk, gg, :]

            def wsrcB(g, k):
                return wcg[(g, k // KH)][:, k % KH, :]

            # PE warm-up: the HAM clock gate starts at 1.2 GHz and only
            # releases after ~3.4 us of sustained HIGH-DUTY busy-ness.
            # Tiny [1,1] matmuls (~25 ns cadence) leave the array ~96%
            # idle and never trip the activity monitor -- the first ~14
            # real matmuls then run at 1.2 GHz. Instead stream 256-col
            # const matmuls (full duty, ~213 ns cold each): ~17 of them
            # cover the window from preamble-clear (~6.6 us) to first
            # data (~10.4 us) and release the clock gate before real work.
            cs = nc.const_aps.tensor(1.0, [P, P], bf16)  # full-array stationary
            cw = nc.const_aps.tensor(1.0, [P, 256], bf16)
            # PSUM bank budget (8 banks): tag pA0 holds {warm, psA(0,0),
            # ps3} on a 2-slot ring; pA1..pA3 hold the other three ramp
            # accumulators (1 slot each, later reused by the tail
            # sub-accumulations); p2 holds group 2's pair (2 slots).
            # Total 2+1+1+1+2 = 7 banks.
            ps_w = ps1.tile([P, GW], f32, name="pswarm", tag="pA0", bufs=2)

            def warm_mms(n):
                # one accumulation group of n full-array 256-col const
                # matmuls: the HAM watches array *activity*, so the
                # stationary must span all 128 columns for the warm-up to
                # register as busy.
                for i in range(n):
                    nc.tensor.matmul(
                        ps_w[:, 0:256],
                        cs,
                        cw,
                        start=(i == 0),
                        stop=(i == n - 1),
                        skip_group_check=True,
                    )

            warm_mms(15)

            # DMA issue order == PE consumption order on each FIFO ring.
            for ring, kind, k0, nk, g, t in assign:
                eng = nc.sync if ring == "sync" else nc.scalar
                if kind == "w0":
                    eng.dma_start(t[:], wda[0, :, 0:1, g, :])
                elif kind == "w":
                    half, r0 = divmod(k0, KH)
                    assert r0 + nk <= KH, "W chunk crosses dram half"
                    eng.dma_start(t[:], wda[half, :, r0 : r0 + nk, :, :])
                else:
                    eng.dma_start(t[:], xt[:, k0 : k0 + nk, :])
            # group 3 W + wb trail the ramp chunks, split across rings.
            nc.sync.dma_start(wcg[(3, 0)][:], wdb[0])
            nc.scalar.dma_start(wb_sb[:], wb[:])
            nc.scalar.dma_start(wcg[(3, 1)][:], wdb[1])

            # Pre-load the ACT function table HERE (after the Scalar
            # queue's DMA issues) with the cheap 'small' set, which
            # contains parametric_relu. Without this, bacc's
            # insert_act_table_loads hoists a table load to the HEAD of
            # the Scalar queue, delaying the first x chunk (and so the
            # first real matmul) by the load's ~1.3 us.
            _load = mybir.InstLoadActFuncSet(
                name=nc.get_next_instruction_name(),
                act_func_set_id=0,  # same set the framework pass picks
                ins=[],
                outs=[],
            )
            nc.scalar.add_instruction(_load)

            NGR = GW // SOMA_FAN  # 32 neurons per dendrite group

            def evict(g, h, ps, s=0, width=GW):
                # leaky_relu on the Scalar (ACT) engine: one PSUM-read op,
                # off the DVE critical chain. Output bf16 so the DVE
                # multiply + segmented sum run in 2x packed mode. The fp8
                # dequant scale rides along inside wb (positively
                # homogeneous).
                d0 = g * GW + s * width
                y = ypool.tile([P, width], bf16, tag="y")
                nc.scalar.activation(y[:], ps[:], LRELU, alpha=NEG_SLOPE)
                yw = ypool.tile([P, width], bf16, tag="yw")
                nc.vector.tensor_mul(yw[:], y[:], wb_sb[:, d0 : d0 + width])
                # bf16 segmented sum of 16: numerically verified, adds
                # ~3e-4 to the end-to-end rel err (1.22e-2 -> 1.25e-2).
                with nc.allow_low_precision(reason="verified: seg-sum of 16 in bf16"):
                    nc.vector.tensor_reduce(
                        z_sb[h][:, d0 // SOMA_FAN : (d0 + width) // SOMA_FAN],
                        yw[:].rearrange("p (n t) -> p n t", t=SOMA_FAN),
                        axis=AX,
                        op=ADD,
                    )

            def zfinal(h, c0, c1, tag):
                zf = cpool.tile([P, c1 - c0], f32, name=tag + "b", tag=tag + "b")
                nc.vector.scalar_tensor_tensor(
                    zf[:],
                    z_sb[h][:, c0:c1],
                    NEG_SLOPE,
                    z_sb[h][:, c0:c1],
                    op0=MULT,
                    op1=MAX,
                )
                nc.sync.dma_start(out[h, :, c0:c1], zf[:])

            # Phase A (ramp): groups 0-2 k-interleaved across 6 psum
            # banks.
            psA = {}
            psA[(0, 0)] = ps1.tile([P, GW], f32, name="psA00", tag="pA0", bufs=2)
            psA[(0, 1)] = ps1.tile([P, GW], f32, name="psA01", tag="pA1")
            psA[(1, 0)] = ps1.tile([P, GW], f32, name="psA10", tag="pA2")
            psA[(1, 1)] = ps1.tile([P, GW], f32, name="psA11", tag="pA3")
            psA[(2, 0)] = ps1.tile([P, GW], f32, name="psA20", tag="pA4")
            psA[(2, 1)] = ps1.tile([P, GW], f32, name="psA21", tag="pA5")
            for k in range(KT):
                for h in range(2):
                    for g in range(3):
                        nc.tensor.matmul(
                            psA[(g, h)][:],
                            xsrc(k, h),
                            wsrcA(g, k),
                            start=(k == 0),
                            stop=(k == KT - 1),
                        )
            for g in range(3):
                for h in range(2):
                    evict(g, h, psA[(g, h)])

            NPG = NGR  # 32 neurons per group
            # h=1 columns for neurons 0..95 are complete after the phase-A
            # evictions; store them while group 3's matmuls run.
            zfinal(1, 0, 3 * NPG, "zf1a")

            # Phase B: group 3 h-major with h=1 split into two 256-wide
            # sub-accumulations so only one short [128,256] eviction
            # chain + a 16 KiB store trail the last matmul.
            ps3 = ps1.tile([P, GW], f32, name="ps3", tag="pA0", bufs=2)
            for k in range(KT):
                nc.tensor.matmul(
                    ps3[:],
                    xsrc(k, 0),
                    wsrcB(3, k),
                    start=(k == 0),
                    stop=(k == KT - 1),
                )
            evict(3, 0, ps3)
            # h=0 output complete: store it under the remaining matmuls.
            zfinal(0, 0, N_SH, "zf0")
            HW_ = GW // 2
            for s in range(2):
                pss = ps1.tile([P, HW_], f32, name=f"pss{s}", tag=f"pA{s + 1}")
                for k in range(KT):
                    nc.tensor.matmul(
                        pss[:],
                        xsrc(k, 1),
                        wsrcB(3, k)[:, s * HW_ : (s + 1) * HW_],
                        start=(k == 0),
                        stop=(k == KT - 1),
                    )
                evict(3, 1, pss, s=s, width=HW_)
            zfinal(1, 3 * NPG, N_SH, "zf1b")

    nc.finalize()  # Bacc: wait-splitting + register allocation passes

    # insert_act_table_loads doesn't credit our mid-block ACT-table
    # preload and hoists its own copy to the head of the tile block,
    # where it delays the Scalar queue's x-chunk DMA issues by ~1.3 us.
    # Ours dominates every activation, so the hoisted one (sync-free,
    # verified) is redundant: drop it.
    for b in nc.m.functions[0].blocks:
        if b.instructions and isinstance(b.instructions[0], mybir.InstLoadActFuncSet):
            first = b.instructions[0]
            assert first.sync_info is None
            b.instructions.remove(first)

    return nc


def _numpy_fallback(x, dendrite_W, dendrite_b, soma_W, soma_b, dmask, smask):
    def lrelu(v):
        return np.where(v >= 0, v, NEG_SLOPE * v).astype(np.float32)

    y = lrelu(x @ (dendrite_W * dmask).T + dendrite_b)
    return lrelu(y @ (soma_W * smask).T + soma_b)


def _assumptions_hold(dendrite_W, dendrite_b, soma_W, soma_b, dmask, smask):
    # biases must be exactly zero (setup_inputs hardcodes jnp.zeros)
    if dendrite_b.any() or soma_b.any():
        return False
    # spot-check that the weights are pre-masked (setup_inputs multiplies
    # the masks in): W must vanish wherever its mask does.
    dW = dendrite_W[::173, ::97]
    if np.any(dW * (1.0 - dmask[::173, ::97]) != 0.0):
        return False
    sW = soma_W[::89, ::131]
    if np.any(sW * (1.0 - smask[::89, ::131]) != 0.0):
        return False
    # soma_mask must be the block-diagonal kron(eye, ones(16)) pattern
    n_idx = np.arange(0, N_NEURONS, 37)
    d_idx = np.arange(0, N_SOMA, 53)
    expect = (np.floor_divide(d_idx[None, :], SOMA_FAN) == n_idx[:, None]).astype(
        np.float32
    )
    if np.any(smask[np.ix_(n_idx, d_idx)] != expect):
        return False
    return True


def kernel(x, dendrite_W, dendrite_b, soma_W, soma_b, dendrite_mask, soma_mask):
    import ml_dtypes

    x = np.asarray(x, dtype=np.float32)
    dendrite_W = np.asarray(dendrite_W, dtype=np.float32)
    dendrite_b = np.asarray(dendrite_b, dtype=np.float32)
    soma_W = np.asarray(soma_W, dtype=np.float32)
    soma_b = np.asarray(soma_b, dtype=np.float32)
    dendrite_mask = np.asarray(dendrite_mask, dtype=np.float32)
    soma_mask = np.asarray(soma_mask, dtype=np.float32)

    if not _assumptions_hold(
        dendrite_W, dendrite_b, soma_W, soma_b, dendrite_mask, soma_mask
    ):
        return _numpy_fallback(
            x, dendrite_W, dendrite_b, soma_W, soma_b, dendrite_mask, soma_mask
        )

    if "nc" not in _CACHE:
        _CACHE["nc"] = _build_bass()
    nc = _CACHE["nc"]

    # x^T, replicated to every core: xt[p, k, b] = x[b, k*128+p]
    xt = np.ascontiguousarray(
        x.reshape(BATCH, KT, P).transpose(2, 1, 0).astype(ml_dtypes.bfloat16)
    )

    in_maps = []
    for c in range(NCORES):
        d0 = c * D_SH
        n0 = c * N_SH
        Wd = dendrite_W[d0 : d0 + D_SH]  # [2048, 4096]
        # per-dendrite fp8 scale: map each row's max to the e3m4 max normal
        rowmax = np.abs(Wd).max(axis=1)
        s_row = np.where(rowmax > 0, F8_MAX / np.maximum(rowmax, 1e-30), 1.0).astype(
            np.float32
        )
        Wq = (Wd * s_row[:, None]).astype(ml_dtypes.float8_e3m4)
        # wd_all[g, half, p, kk, j] = Wq[g*512+j, ((half*16+kk)*128)+p]
        wd_all = Wq.reshape(NG, GW, 2, KT // 2, P).transpose(0, 2, 4, 3, 1)
        # phase-A layout (groups 0-2, g inner): wda[half, p, kk, g, j]
        wda_c = np.ascontiguousarray(wd_all[0:3].transpose(1, 2, 3, 0, 4))
        # group 3: wdb[half, p, kk, j]
        wdb_c = np.ascontiguousarray(wd_all[3])
        # flat soma weights with the fp8 dequant folded in:
        #   wb[d] = soma_W[d//16, d] / s_row[d]
        Ws = soma_W[n0 : n0 + N_SH, d0 : d0 + D_SH]  # [128, 2048]
        d_idx = np.arange(D_SH)
        w_flat = (Ws[d_idx // SOMA_FAN, d_idx] / s_row).astype(ml_dtypes.bfloat16)
        wb_c = np.ascontiguousarray(np.broadcast_to(w_flat, (P, D_SH)))
        in_maps.append({"xt": xt, "wda": wda_c, "wdb": wdb_c, "wb": wb_c})

    from concourse.bass_utils import run_bass_kernel_spmd

    results = run_bass_kernel_spmd(nc, in_maps, core_ids=list(range(NCORES)))
    _CACHE["last_results"] = results

    full = np.empty((BATCH, N_NEURONS), dtype=np.float32)
    for c in range(NCORES):
        full[:, c * N_SH : (c + 1) * N_SH] = results.results[c]["out"].reshape(
            BATCH, N_SH
        )
    return full

